# revision 10
# baseline (speedup 1.0000x reference)
"""Trainium2 Bass kernel for nn_DALayer (moe_routing, squeeze-excite style).

Computation (per sample b):
    y    = mean(x[b], axis=(H,W))                 # [C]
    h    = relu(W1[dataset[b]] @ y)               # [HID]
    gate = sigmoid(W2[dataset[b]] @ h)            # [C]
    out[b] = x[b] * gate[:, None, None]

Sharding: pure data parallel over batch across 8 NeuronCores (8 samples
per core); expert weights replicated.  Single pass over x: each sample's
x stays resident in SBUF between the mean-reduce and the gate multiply,
so HBM traffic per core is 64 MiB in + 64 MiB out — the kernel is purely
HBM-bandwidth-bound (roofline ~375 us at ~358 GB/s per core).

Expert routing is done on-device: all three experts' W1 rows are stacked
([96, C]) so one accumulating matmul chain produces h for every expert;
a mask (dataset broadcast across partitions + per-expert-block is_equal)
zeroes the two unselected experts' h, and a stacked-W2 matmul then
yields the selected expert's gate directly.

Variant knobs (for HW A/B; VARIANT picks the shipped config):
  load_chunks:  4 -> four [128,4096] loads per sample; 1 -> one 8 MiB load
  store_chunks: likewise for stores
  store_ring:   "scalar" (ACT HWDGE ring, concurrent with loads) or
                "sync" (same SP ring as loads -> strict FIFO interleave,
                coarse read/write phase alternation on HBM)
  bufs:         sample-tile pool depth (each buf is 64 KiB/partition)

HW A/B results (interleaved (8,64)-pass slope, min over reps, noise ~+-15us):
  base 388-400us (shipped) < coarse 406 < base_il ~409 < base_sync 416
  < mix 420 < deep 429.  Fine 2 MiB load/store interleave on separate
  HWDGE rings with tight bufs=2 WAR coupling beats every coarser or
  more-decoupled schedule tried; ~97% of the ~358 GB/s per-core HBM
  roofline (134 MiB/pass -> 375us ideal).
"""

import os

import numpy as np
from contextlib import ExitStack

import concourse.tile as tile
from concourse import bacc, mybir
from concourse import bass_utils

# Problem shapes (hardcoded per contract).
B, C, H, W = 64, 512, 64, 64
HW = H * W                 # 4096 spatial elements
N_CORES = 8
BL = B // N_CORES          # 8 samples per core
NE, HID = 3, 32
M96 = NE * HID             # 96 stacked expert-hidden rows
P = 128                    # SBUF partitions
J = C // P                 # 4 channel chunks of 128

# (load_chunks, store_chunks, store_ring, bufs)
VARIANTS = {
    "base":      dict(load_chunks=4, store_chunks=4, store_ring="scalar", bufs=2),
    "deep":      dict(load_chunks=4, store_chunks=4, store_ring="scalar", bufs=3),
    "mix":       dict(load_chunks=4, store_chunks=1, store_ring="scalar", bufs=3),
    "mix_sync":  dict(load_chunks=4, store_chunks=1, store_ring="sync",   bufs=3),
    "coarse":    dict(load_chunks=1, store_chunks=1, store_ring="scalar", bufs=3),
    "coarse_sync": dict(load_chunks=1, store_chunks=1, store_ring="sync", bufs=3),
    "base_sync": dict(load_chunks=4, store_chunks=4, store_ring="sync",   bufs=3),
    "base_il":   dict(load_chunks=4, store_chunks=4, store_ring="scalar", bufs=2, il=True),
    "coarse2":   dict(load_chunks=1, store_chunks=1, store_ring="scalar", bufs=2),
    "half_il":   dict(load_chunks=2, store_chunks=2, store_ring="scalar", bufs=2, il=True),
}
VARIANT = "base"

_nc_cache = {}


def _build(passes=1, variant=None):
    """Build + compile the per-core Bass module (cached).

    passes>1 repeats the whole pipeline (for timing: slope across pass
    counts cancels fixed dispatch overhead)."""
    variant = variant or VARIANT
    key = (passes, variant)
    if key in _nc_cache:
        return _nc_cache[key]
    cfg = VARIANTS[variant]
    LCH, SCH, SRING, BUFS = (
        cfg["load_chunks"], cfg["store_chunks"], cfg["store_ring"], cfg["bufs"],
    )
    IL = cfg.get("il", False)

    f32 = mybir.dt.float32
    i32 = mybir.dt.int32
    FT = mybir.ActivationFunctionType

    nc = bacc.Bacc(
        "TRN2",
        target_bir_lowering=False,
        debug=False,
        enable_asserts=False,
        num_devices=N_CORES,
    )
    x = nc.dram_tensor("x", [BL, C, H, W], f32, kind="ExternalInput").ap()
    d = nc.dram_tensor("d", [1, BL], i32, kind="ExternalInput").ap()
    w1t = nc.dram_tensor("w1t", [C, M96], f32, kind="ExternalInput").ap()
    w2t = nc.dram_tensor("w2t", [M96, C], f32, kind="ExternalInput").ap()
    out = nc.dram_tensor("out", [BL, C, H, W], f32, kind="ExternalOutput").ap()

    xr = x.rearrange("b c h w -> b c (h w)")          # [BL, C, HW]
    outr = out.rearrange("b c h w -> b c (h w)")
    # [BL, P, J, HW] views for single-DMA whole-sample transfers; partition
    # dim must lead on both sides (index-aligned [p, j, s] <-> SBUF [p,(j s)])
    xj = xr.rearrange("b (j p) s -> b p j s", p=P)
    outj = outr.rearrange("b (j p) s -> b p j s", p=P)

    store_engine_attr = "scalar" if SRING == "scalar" else "sync"

    with ExitStack() as ctx:
        tc = ctx.enter_context(tile.TileContext(nc))
        const = ctx.enter_context(tc.tile_pool(name="const", bufs=1))
        xpool = ctx.enter_context(tc.tile_pool(name="xp", bufs=BUFS))
        small = ctx.enter_context(tc.tile_pool(name="small", bufs=4))
        ps_h = ctx.enter_context(tc.tile_pool(name="psh", bufs=2, space="PSUM"))
        ps_g = ctx.enter_context(tc.tile_pool(name="psg", bufs=2, space="PSUM"))

        # ---- weights / routing constants (tiny, loaded once) ----
        # w1_sb columns [96j, 96j+96) hold chunk j: lhsT [K=128 c, M=96 (e,hid)]
        w1_sb = const.tile([P, J * M96], f32)
        for j in range(J):
            nc.sync.dma_start(w1_sb[:, j * M96:(j + 1) * M96], w1t[j * P:(j + 1) * P, :])
        w2_sb = const.tile([M96, C], f32)       # lhsT [K=96, M=128] per c-chunk
        nc.sync.dma_start(w2_sb[:], w2t)
        # dataset replicated across 96 partitions (stride-0 DMA read), cast,
        # then mask[32e+k, b] = (dataset[b] == e) built per 32-aligned block
        di_bc = const.tile([M96, BL], i32)
        nc.sync.dma_start(di_bc[:], d.broadcast_to([M96, BL]))
        df_bc = const.tile([M96, BL], f32)
        nc.vector.tensor_copy(df_bc[:], di_bc[:])          # int32 -> f32 cast
        m_sb = const.tile([M96, BL], f32)
        for e in range(NE):
            nc.vector.tensor_scalar(
                m_sb[e * HID:(e + 1) * HID, :], df_bc[e * HID:(e + 1) * HID, :],
                float(e), None, op0=mybir.AluOpType.is_equal,
            )

        # ---- per-sample pipeline ----
        store_dma = getattr(nc, store_engine_attr).dma_start

        SW = J // SCH                      # j-chunks per store piece

        def emit_store_piece(b, xt, g):
            if SW == J:
                store_dma(outj[b], xt[:].rearrange("p (j s) -> p j s", j=J))
            elif SW == 1:
                store_dma(outr[b, g * P:(g + 1) * P, :], xt[:, g * HW:(g + 1) * HW])
            else:
                j0 = g * SW
                store_dma(outj[b][:, j0:j0 + SW, :],
                          xt[:].rearrange("p (j s) -> p j s", j=J)[:, j0:j0 + SW, :])

        def emit_store(b, xt):
            for g in range(SCH):
                emit_store_piece(b, xt, g)

        # On the shared SP ring, a store's mul-done wait would head-of-line
        # block the next loads, so delay its ISSUE by store_shift samples
        # (software pipelining of the ring order).  On the independent ACT
        # ring, issue immediately after the muls.
        store_shift = (BUFS - 1) if SRING == "sync" else 0
        pending = []

        for b in [bb for _ in range(passes) for bb in range(BL)]:
            # one [128, J*HW] tile per sample; column block j = channel chunk j
            xt = xpool.tile([P, J * HW], f32, tag="xt")
            if LCH == 1:
                nc.sync.dma_start(xt[:].rearrange("p (j s) -> p j s", j=J), xj[b])
            elif LCH == J:
                for j in range(J):
                    nc.sync.dma_start(xt[:, j * HW:(j + 1) * HW], xr[b, j * P:(j + 1) * P, :])
            else:
                LW = J // LCH
                for g in range(LCH):
                    j0 = g * LW
                    nc.sync.dma_start(
                        xt[:].rearrange("p (j s) -> p j s", j=J)[:, j0:j0 + LW, :],
                        xj[b][:, j0:j0 + LW, :])
            # store(k) must be emitted before load(k+BUFS) (same-ring WAR on
            # the pool slot), i.e. at latest right after loads(k+BUFS-1)
            if store_shift and len(pending) >= store_shift:
                emit_store(*pending.pop(0))
            # channel sums (mean * HW); scale folded into the relu below
            ysum = small.tile([P, J], f32, tag="y")
            for j in range(J):
                nc.vector.tensor_reduce(
                    ysum[:, j:j + 1], xt[:, j * HW:(j + 1) * HW],
                    axis=mybir.AxisListType.X, op=mybir.AluOpType.add,
                )
            # h for all 3 experts at once: [96, 1]
            h_ps = ps_h.tile([M96, 1], f32, tag="h")
            for j in range(J):
                nc.tensor.matmul(
                    h_ps[:], w1_sb[:, j * M96:(j + 1) * M96], ysum[:, j:j + 1],
                    start=(j == 0), stop=(j == J - 1),
                )
            h_sb = small.tile([M96, 1], f32, tag="hs")
            nc.scalar.activation(h_sb[:], h_ps[:], FT.Relu, scale=1.0 / HW)
            hm_sb = small.tile([M96, 1], f32, tag="hm")
            nc.vector.tensor_mul(hm_sb[:], h_sb[:], m_sb[:, b:b + 1])
            # gate[c] for the selected expert, c-chunk j in column j
            g_ps = ps_g.tile([P, J], f32, tag="g")
            for j in range(J):
                nc.tensor.matmul(
                    g_ps[:, j:j + 1], w2_sb[:, j * P:(j + 1) * P], hm_sb[:],
                    start=True, stop=True,
                )
            g_sb = small.tile([P, J], f32, tag="gs")
            nc.scalar.activation(g_sb[:], g_ps[:], FT.Sigmoid)
            # apply gate in place; il=True releases each chunk's store right
            # after its mul (earlier WAR release for the b+BUFS load)
            for j in range(J):
                nc.scalar.mul(xt[:, j * HW:(j + 1) * HW], xt[:, j * HW:(j + 1) * HW],
                              g_sb[:, j:j + 1])
                if IL and (j + 1) % SW == 0:
                    emit_store_piece(b, xt, (j + 1) // SW - 1)
            if IL:
                continue
            pending.append((b, xt))
            if store_shift == 0:
                emit_store(*pending.pop(0))
        while pending:
            emit_store(*pending.pop(0))

    nc.compile()
    _nc_cache[key] = nc
    return nc


def _prep_shared(W1, W2):
    # lhsT layouts: w1t[c, 32e+k] = W1[e, k, c]; w2t[32e+k, c] = W2[e, c, k]
    w1t = np.ascontiguousarray(W1.transpose(2, 0, 1).reshape(C, M96)).astype(np.float32, copy=False)
    w2t = np.ascontiguousarray(W2.transpose(0, 2, 1).reshape(M96, C)).astype(np.float32, copy=False)
    return w1t, w2t


def _core_inputs(x_sl, d_sl, w1t, w2t):
    return {
        "x": x_sl,
        "d": np.ascontiguousarray(np.asarray(d_sl).reshape(1, BL)),
        "w1t": w1t,
        "w2t": w2t,
    }


def kernel(x, dataset, W1, W2):
    # NTFF tracing is unavailable under this axon client (antenv.axon_hooks
    # missing); make sure an inherited BASS_TRACE can't divert us into it.
    os.environ["BASS_NEVER_TRACE"] = "1"
    nc = _build()
    x = np.ascontiguousarray(np.asarray(x, dtype=np.float32))
    w1t, w2t = _prep_shared(np.asarray(W1), np.asarray(W2))
    dataset = np.asarray(dataset, dtype=np.int32)
    in_maps = []
    for c in range(N_CORES):
        sl = slice(c * BL, (c + 1) * BL)
        in_maps.append(_core_inputs(x[sl], dataset[sl], w1t, w2t))
    res = bass_utils.run_bass_kernel_spmd(
        nc, in_maps, core_ids=list(range(N_CORES)),
    )
    return np.concatenate([r["out"] for r in res.results], axis=0)


# revision 13
# speedup vs baseline: 1.0000x; 1.0000x over previous
"""Trainium2 Bass kernel for nn_DALayer (moe_routing, squeeze-excite style).

Computation (per sample b):
    y    = mean(x[b], axis=(H,W))                 # [C]
    h    = relu(W1[dataset[b]] @ y)               # [HID]
    gate = sigmoid(W2[dataset[b]] @ h)            # [C]
    out[b] = x[b] * gate[:, None, None]

Sharding: pure data parallel over batch across 8 NeuronCores (8 samples
per core); expert weights replicated.  Single pass over x: each sample's
x stays resident in SBUF between the mean-reduce and the gate multiply,
so HBM traffic per core is 64 MiB in + 64 MiB out — the kernel is purely
HBM-bandwidth-bound (roofline ~375 us at ~358 GB/s per core).

Expert routing is done on-device: all three experts' W1 rows are stacked
([96, C]) so one accumulating matmul chain produces h for every expert;
a mask (dataset broadcast across partitions + per-expert-block is_equal)
zeroes the two unselected experts' h, and a stacked-W2 matmul then
yields the selected expert's gate directly.

Variant knobs (for HW A/B; VARIANT picks the shipped config):
  load_chunks:  4 -> four [128,4096] loads per sample; 1 -> one 8 MiB load
  store_chunks: likewise for stores
  store_ring:   "scalar" (ACT HWDGE ring, concurrent with loads) or
                "sync" (same SP ring as loads -> strict FIFO interleave,
                coarse read/write phase alternation on HBM)
  bufs:         sample-tile pool depth (each buf is 64 KiB/partition)

HW A/B results (interleaved (8,64)-pass slope, min over reps, noise ~+-15us):
  base 388-400us (shipped) < coarse 406 < half ~408 < base_il ~409 <
  fine ~413 < base_sync 416 < mix 420 < deep 429.  Transfer-size sweep:
  1 MiB 413+ / 2 MiB 388-400 / 4 MiB 408+ / 8 MiB 406+ -> 2 MiB chunks on
  separate HWDGE rings with tight bufs=2 WAR coupling beat every coarser,
  finer, or more-decoupled schedule tried; ~97% of the ~358 GB/s per-core
  HBM roofline (134 MiB/pass -> 375us ideal).
"""

import os

import numpy as np
from contextlib import ExitStack

import concourse.tile as tile
from concourse import bacc, mybir
from concourse import bass_utils

# Problem shapes (hardcoded per contract).
B, C, H, W = 64, 512, 64, 64
HW = H * W                 # 4096 spatial elements
N_CORES = 8
BL = B // N_CORES          # 8 samples per core
NE, HID = 3, 32
M96 = NE * HID             # 96 stacked expert-hidden rows
P = 128                    # SBUF partitions
J = C // P                 # 4 channel chunks of 128

# (load_chunks, store_chunks, store_ring, bufs)
VARIANTS = {
    "base":      dict(load_chunks=4, store_chunks=4, store_ring="scalar", bufs=2),
    "deep":      dict(load_chunks=4, store_chunks=4, store_ring="scalar", bufs=3),
    "mix":       dict(load_chunks=4, store_chunks=1, store_ring="scalar", bufs=3),
    "mix_sync":  dict(load_chunks=4, store_chunks=1, store_ring="sync",   bufs=3),
    "coarse":    dict(load_chunks=1, store_chunks=1, store_ring="scalar", bufs=3),
    "coarse_sync": dict(load_chunks=1, store_chunks=1, store_ring="sync", bufs=3),
    "base_sync": dict(load_chunks=4, store_chunks=4, store_ring="sync",   bufs=3),
    "base_il":   dict(load_chunks=4, store_chunks=4, store_ring="scalar", bufs=2, il=True),
    "coarse2":   dict(load_chunks=1, store_chunks=1, store_ring="scalar", bufs=2),
    "half_il":   dict(load_chunks=2, store_chunks=2, store_ring="scalar", bufs=2, il=True),
    "half":      dict(load_chunks=2, store_chunks=2, store_ring="scalar", bufs=2),
    "fine":      dict(load_chunks=8, store_chunks=8, store_ring="scalar", bufs=2),
}
VARIANT = "base"

_nc_cache = {}


def _build(passes=1, variant=None):
    """Build + compile the per-core Bass module (cached).

    passes>1 repeats the whole pipeline (for timing: slope across pass
    counts cancels fixed dispatch overhead)."""
    variant = variant or VARIANT
    key = (passes, variant)
    if key in _nc_cache:
        return _nc_cache[key]
    cfg = VARIANTS[variant]
    LCH, SCH, SRING, BUFS = (
        cfg["load_chunks"], cfg["store_chunks"], cfg["store_ring"], cfg["bufs"],
    )
    IL = cfg.get("il", False)

    f32 = mybir.dt.float32
    i32 = mybir.dt.int32
    FT = mybir.ActivationFunctionType

    nc = bacc.Bacc(
        "TRN2",
        target_bir_lowering=False,
        debug=False,
        enable_asserts=False,
        num_devices=N_CORES,
    )
    x = nc.dram_tensor("x", [BL, C, H, W], f32, kind="ExternalInput").ap()
    d = nc.dram_tensor("d", [1, BL], i32, kind="ExternalInput").ap()
    w1t = nc.dram_tensor("w1t", [C, M96], f32, kind="ExternalInput").ap()
    w2t = nc.dram_tensor("w2t", [M96, C], f32, kind="ExternalInput").ap()
    out = nc.dram_tensor("out", [BL, C, H, W], f32, kind="ExternalOutput").ap()

    xr = x.rearrange("b c h w -> b c (h w)")          # [BL, C, HW]
    outr = out.rearrange("b c h w -> b c (h w)")
    # [BL, P, J, HW] views for single-DMA whole-sample transfers; partition
    # dim must lead on both sides (index-aligned [p, j, s] <-> SBUF [p,(j s)])
    xj = xr.rearrange("b (j p) s -> b p j s", p=P)
    outj = outr.rearrange("b (j p) s -> b p j s", p=P)

    store_engine_attr = "scalar" if SRING == "scalar" else "sync"

    with ExitStack() as ctx:
        tc = ctx.enter_context(tile.TileContext(nc))
        const = ctx.enter_context(tc.tile_pool(name="const", bufs=1))
        xpool = ctx.enter_context(tc.tile_pool(name="xp", bufs=BUFS))
        small = ctx.enter_context(tc.tile_pool(name="small", bufs=4))
        ps_h = ctx.enter_context(tc.tile_pool(name="psh", bufs=2, space="PSUM"))
        ps_g = ctx.enter_context(tc.tile_pool(name="psg", bufs=2, space="PSUM"))

        # ---- weights / routing constants (tiny, loaded once) ----
        # w1_sb columns [96j, 96j+96) hold chunk j: lhsT [K=128 c, M=96 (e,hid)]
        w1_sb = const.tile([P, J * M96], f32)
        for j in range(J):
            nc.sync.dma_start(w1_sb[:, j * M96:(j + 1) * M96], w1t[j * P:(j + 1) * P, :])
        w2_sb = const.tile([M96, C], f32)       # lhsT [K=96, M=128] per c-chunk
        nc.sync.dma_start(w2_sb[:], w2t)
        # dataset replicated across 96 partitions (stride-0 DMA read), cast,
        # then mask[32e+k, b] = (dataset[b] == e) built per 32-aligned block
        di_bc = const.tile([M96, BL], i32)
        nc.sync.dma_start(di_bc[:], d.broadcast_to([M96, BL]))
        df_bc = const.tile([M96, BL], f32)
        nc.vector.tensor_copy(df_bc[:], di_bc[:])          # int32 -> f32 cast
        m_sb = const.tile([M96, BL], f32)
        for e in range(NE):
            nc.vector.tensor_scalar(
                m_sb[e * HID:(e + 1) * HID, :], df_bc[e * HID:(e + 1) * HID, :],
                float(e), None, op0=mybir.AluOpType.is_equal,
            )

        # ---- per-sample pipeline ----
        store_dma = getattr(nc, store_engine_attr).dma_start

        SW = max(J // SCH, 1)              # j-chunks per store piece
        SF = max(SCH // J, 1)              # fragments per j-chunk (SCH > J)
        SFW = HW // SF

        def emit_store_piece(b, xt, g):
            if SF > 1:
                j, f = divmod(g, SF)
                c0 = j * HW + f * SFW
                store_dma(outr[b, j * P:(j + 1) * P, f * SFW:(f + 1) * SFW],
                          xt[:, c0:c0 + SFW])
            elif SW == J:
                store_dma(outj[b], xt[:].rearrange("p (j s) -> p j s", j=J))
            elif SW == 1:
                store_dma(outr[b, g * P:(g + 1) * P, :], xt[:, g * HW:(g + 1) * HW])
            else:
                j0 = g * SW
                store_dma(outj[b][:, j0:j0 + SW, :],
                          xt[:].rearrange("p (j s) -> p j s", j=J)[:, j0:j0 + SW, :])

        def emit_store(b, xt):
            for g in range(SCH):
                emit_store_piece(b, xt, g)

        # On the shared SP ring, a store's mul-done wait would head-of-line
        # block the next loads, so delay its ISSUE by store_shift samples
        # (software pipelining of the ring order).  On the independent ACT
        # ring, issue immediately after the muls.
        store_shift = (BUFS - 1) if SRING == "sync" else 0
        pending = []

        for b in [bb for _ in range(passes) for bb in range(BL)]:
            # one [128, J*HW] tile per sample; column block j = channel chunk j
            xt = xpool.tile([P, J * HW], f32, tag="xt")
            if LCH > J:
                F = LCH // J
                FW = HW // F
                for j in range(J):
                    for f in range(F):
                        c0 = j * HW + f * FW
                        nc.sync.dma_start(xt[:, c0:c0 + FW],
                                          xr[b, j * P:(j + 1) * P, f * FW:(f + 1) * FW])
            elif LCH == 1:
                nc.sync.dma_start(xt[:].rearrange("p (j s) -> p j s", j=J), xj[b])
            elif LCH == J:
                for j in range(J):
                    nc.sync.dma_start(xt[:, j * HW:(j + 1) * HW], xr[b, j * P:(j + 1) * P, :])
            else:
                LW = J // LCH
                for g in range(LCH):
                    j0 = g * LW
                    nc.sync.dma_start(
                        xt[:].rearrange("p (j s) -> p j s", j=J)[:, j0:j0 + LW, :],
                        xj[b][:, j0:j0 + LW, :])
            # store(k) must be emitted before load(k+BUFS) (same-ring WAR on
            # the pool slot), i.e. at latest right after loads(k+BUFS-1)
            if store_shift and len(pending) >= store_shift:
                emit_store(*pending.pop(0))
            # channel sums (mean * HW); scale folded into the relu below
            ysum = small.tile([P, J], f32, tag="y")
            for j in range(J):
                nc.vector.tensor_reduce(
                    ysum[:, j:j + 1], xt[:, j * HW:(j + 1) * HW],
                    axis=mybir.AxisListType.X, op=mybir.AluOpType.add,
                )
            # h for all 3 experts at once: [96, 1]
            h_ps = ps_h.tile([M96, 1], f32, tag="h")
            for j in range(J):
                nc.tensor.matmul(
                    h_ps[:], w1_sb[:, j * M96:(j + 1) * M96], ysum[:, j:j + 1],
                    start=(j == 0), stop=(j == J - 1),
                )
            h_sb = small.tile([M96, 1], f32, tag="hs")
            nc.scalar.activation(h_sb[:], h_ps[:], FT.Relu, scale=1.0 / HW)
            hm_sb = small.tile([M96, 1], f32, tag="hm")
            nc.vector.tensor_mul(hm_sb[:], h_sb[:], m_sb[:, b:b + 1])
            # gate[c] for the selected expert, c-chunk j in column j
            g_ps = ps_g.tile([P, J], f32, tag="g")
            for j in range(J):
                nc.tensor.matmul(
                    g_ps[:, j:j + 1], w2_sb[:, j * P:(j + 1) * P], hm_sb[:],
                    start=True, stop=True,
                )
            g_sb = small.tile([P, J], f32, tag="gs")
            nc.scalar.activation(g_sb[:], g_ps[:], FT.Sigmoid)
            # apply gate in place; il=True releases each chunk's store right
            # after its mul (earlier WAR release for the b+BUFS load)
            for j in range(J):
                nc.scalar.mul(xt[:, j * HW:(j + 1) * HW], xt[:, j * HW:(j + 1) * HW],
                              g_sb[:, j:j + 1])
                if IL and (j + 1) % SW == 0:
                    emit_store_piece(b, xt, (j + 1) // SW - 1)
            if IL:
                continue
            pending.append((b, xt))
            if store_shift == 0:
                emit_store(*pending.pop(0))
        while pending:
            emit_store(*pending.pop(0))

    nc.compile()
    _nc_cache[key] = nc
    return nc


def _prep_shared(W1, W2):
    # lhsT layouts: w1t[c, 32e+k] = W1[e, k, c]; w2t[32e+k, c] = W2[e, c, k]
    w1t = np.ascontiguousarray(W1.transpose(2, 0, 1).reshape(C, M96)).astype(np.float32, copy=False)
    w2t = np.ascontiguousarray(W2.transpose(0, 2, 1).reshape(M96, C)).astype(np.float32, copy=False)
    return w1t, w2t


def _core_inputs(x_sl, d_sl, w1t, w2t):
    return {
        "x": x_sl,
        "d": np.ascontiguousarray(np.asarray(d_sl).reshape(1, BL)),
        "w1t": w1t,
        "w2t": w2t,
    }


def kernel(x, dataset, W1, W2):
    # NTFF tracing is unavailable under this axon client (antenv.axon_hooks
    # missing); make sure an inherited BASS_TRACE can't divert us into it.
    os.environ["BASS_NEVER_TRACE"] = "1"
    nc = _build()
    x = np.ascontiguousarray(np.asarray(x, dtype=np.float32))
    w1t, w2t = _prep_shared(np.asarray(W1), np.asarray(W2))
    dataset = np.asarray(dataset, dtype=np.int32)
    in_maps = []
    for c in range(N_CORES):
        sl = slice(c * BL, (c + 1) * BL)
        in_maps.append(_core_inputs(x[sl], dataset[sl], w1t, w2t))
    res = bass_utils.run_bass_kernel_spmd(
        nc, in_maps, core_ids=list(range(N_CORES)),
    )
    return np.concatenate([r["out"] for r in res.results], axis=0)
